# revision 35
# baseline (speedup 1.0000x reference)
"""Photonic-mesh (NEUROPULS) chain kernel for Trainium2, 8 NeuronCores.

Group-decomposition scheme (2 launches):

Launch A (8 cores, group-parallel): the 128 fused C-stages are split into 8
groups of 16.  Core g builds its group's transfer operator T_g on (E,O)
pair-space in a banded window representation [128 pairs, W cols],
w = c - 2j + b (b=35, W=74): per stage the same phase1 (4 CMULA + 2 adds)
+ crossing (2 fp16 shift matmuls + 2 signed-add CMULA) as the sequential
kernel, but on the identity's band instead of problem columns, and only 16
sequential stages instead of 128.  Group 7's last stage (half-epilogue) gets
zero crossing weights via its in_map.

Host between launches: pure reindexing - window -> dense [256,256], parity
block split, transpose, fp16 cast -> matmul weights.

Launch B (8 cores, column-parallel): each core chain-applies T_0..T_7 to its
16 initial columns: per level 12 fp16 PE matmuls (complex product via real
part + sign-split imag halves) accumulating in PSUM, then the projection.
"""

import math

import numpy as np

import concourse.bass as bass
import concourse.mybir as mybir
from concourse.ap import AP

N = 128
NCORES = 8
COLS = N // NCORES  # 16 columns per core in launch B
NSTAGES = 129       # 128 C-type stages (h0 + 126 full + half-epi) + projection
KSTG = 16           # C-stages per group
WW = 74             # window width (complex cols) in launch A
B0 = 35             # window center: w = c - 2j + B0

IL_MMI = 0.05
IMB = 0.005
IL_X = 0.02
CT = 0.01

F32 = mybir.dt.float32
F16 = mybir.dt.float16
WT16 = float(np.float64(np.float16(math.sqrt(1.0 - CT) / math.sqrt(CT))))

# ----------------------------------------------------------------------------
# custom DVE op: out[p,s,k] = in1[p,s,k]*s0[p] + in0[p,s,k]*s1[p]*(2s-1)
# ----------------------------------------------------------------------------
_CMULA = None


def _get_cmula():
    global _CMULA
    if _CMULA is not None:
        return _CMULA
    import concourse.dve_ops as dom
    from concourse.dve_ops import OPS, DveOp
    from concourse.dve_spec import Spec, Src0, Src1, C0, C1, SubIdx, One, lower
    from concourse.dve_uop import DveOpSpec

    name = "CMULA_NP_ANT"
    for op in OPS:  # idempotent across re-imports
        if op.name == name:
            _CMULA = op
            return op

    def _ref(in0, in1, s0, s1, imm2):
        pg = (np.arange(in0.shape[1], dtype=np.float32) * 2.0 - 1.0).reshape(1, -1, 1)
        a = np.asarray(s0, np.float32).reshape(-1, 1, 1) if np.ndim(s0) else np.float32(s0)
        b = np.asarray(s1, np.float32).reshape(-1, 1, 1) if np.ndim(s1) else np.float32(s1)
        return (np.asarray(in1, np.float32).reshape(in0.shape) * a
                + np.asarray(in0, np.float32) * b * pg).astype(np.float32)

    op = DveOp(
        name,
        Spec(body=Src1 * C0 + Src0 * C1 * (SubIdx + SubIdx - One), reference=_ref),
        subdim=True,
        uops_sha={},
    )
    OPS.append(op)
    dom._SUB_OPCODE_FOR_NAME[name] = dom._CUSTOM_DVE_ROW_BASE + len(OPS) - 1
    dom.CUSTOM_DVE_SPECS[name] = op.spec
    for ver in ("v3", "v4"):
        spec_c = DveOpSpec(name=name, opcode=dom.get_dve_sub_opcode(name),
                           uops=lower(op.spec, ver=ver), rd1_en=True)
        op.uops_sha[ver] = spec_c.sha(ver)
    _CMULA = op
    return op


def _nat(t, w, lo, hi):
    """[P, 2, hi-lo] natural-page view of a [P, 2w] complex-packed AP."""
    return AP(t.tensor, t.offset + lo, [list(t.ap[0]), [w, 2], [1, hi - lo]])


def _swp(t, w, lo, hi):
    """[P, 2, hi-lo] page-swapped view (page0 = imag half)."""
    return AP(t.tensor, t.offset + w + lo, [list(t.ap[0]), [-w, 2], [1, hi - lo]])


# ----------------------------------------------------------------------------
# Launch A: windowed group transfer operators
# ----------------------------------------------------------------------------
_PROG_A = None


def _build_program_a():
    global _PROG_A
    if _PROG_A is not None:
        return _PROG_A
    CMULA = _get_cmula()

    import concourse.bacc as bacc
    nc = bacc.Bacc(None, target_bir_lowering=False)
    d_xe = nc.declare_dram_parameter("xe0", [N, 2 * WW], F32, isOutput=False)
    d_xo = nc.declare_dram_parameter("xo0", [N, 2 * WW], F32, isOutput=False)
    d_coef = [nc.declare_dram_parameter(f"coef{i}", [N, KSTG], F32, isOutput=False)
              for i in range(8)]
    d_sh = nc.declare_dram_parameter("shiftT", [N, 2 * N], F16, isOutput=False)
    d_shl = nc.declare_dram_parameter("shiftTl", [N, 2 * N], F16, isOutput=False)
    d_oE = nc.declare_dram_parameter("outE", [N, 2 * WW], F32, isOutput=True)
    d_oO = nc.declare_dram_parameter("outO", [N, 2 * WW], F32, isOutput=True)

    from concourse import tile

    with tile.TileContext(nc) as tc:
        with (tc.tile_pool(name="const", bufs=1) as cpool,
              tc.tile_pool(name="tmp", bufs=2) as tpool,
              tc.tile_pool(name="ps", bufs=2, space="PSUM") as ppool):
            coefT = cpool.tile([N, 8 * KSTG], F32, tag="coef")
            shT = cpool.tile([N, 2 * N], F16, tag="sh")
            shTl = cpool.tile([N, 2 * N], F16, tag="shl")
            coef = [coefT[:, i * KSTG:(i + 1) * KSTG] for i in range(8)]
            # manually double-buffered state (guard columns must stay zero)
            xeA = cpool.tile([N, 2 * WW], F32, tag="st0", name="xeA")
            xoA = cpool.tile([N, 2 * WW], F32, tag="st1", name="xoA")
            xeB = cpool.tile([N, 2 * WW], F32, tag="st2", name="xeB")
            xoB = cpool.tile([N, 2 * WW], F32, tag="st3", name="xoB")
            nc.vector.memset(xeB[:], 0.0)
            nc.vector.memset(xoB[:], 0.0)
            nc.sync.dma_start(xeA[:], d_xe[:])
            nc.sync.dma_start(xoA[:], d_xo[:])
            for i in range(8):
                nc.sync.dma_start(coef[i], d_coef[i][:])
            nc.sync.dma_start(shT[:], d_sh[:])
            nc.sync.dma_start(shTl[:], d_shl[:])

            def cmul(dst, src, cr, ci, lo, hi):
                # full-width: in1 must stay 2D for AP-scalar coefficients;
                # out-of-band results are garbage over stale state but all
                # downstream consumers slice to [lo, hi) where inputs are valid
                return nc.vector._custom_dve(
                    CMULA, out=_nat(dst[:], WW, 0, WW), in0=_swp(src[:], WW, 0, WW),
                    in1=src[:], s0=cr, s1=ci)

            xe, xo = xeA, xoA
            nxt = [xeB, xoB]
            for k in range(KSTG):
                c = [coef[i][:, k:k + 1] for i in range(8)]
                sh = shTl if k == KSTG - 1 else shT
                up = sh[:, 0:N]
                dn = sh[:, N:2 * N]
                # live band before stage k: [a, b]; phase1 computed on the
                # union read range [a-2, b+3) so the shifted matmul reads and
                # phase2 in1 slices stay inside freshly-written data.
                a = B0 - 2 * k
                b = B0 + 1 + 2 * k
                lo, hi = a - 2, b + 3
                te1 = tpool.tile([N, 2 * WW], F32, tag="te1")
                te2 = tpool.tile([N, 2 * WW], F32, tag="te2")
                to1 = tpool.tile([N, 2 * WW], F32, tag="to1")
                to2 = tpool.tile([N, 2 * WW], F32, tag="to2")
                # O-side first so psB is ready before phase2-xe issues on V
                cmul(to1, xe, c[4], c[5], lo, hi)
                cmul(to2, xo, c[6], c[7], lo, hi)
                cmul(te1, xe, c[0], c[1], lo, hi)
                cmul(te2, xo, c[2], c[3], lo, hi)
                e2 = tpool.tile([N, 2 * WW], F16, tag="e2")
                o2 = tpool.tile([N, 2 * WW], F16, tag="o2")
                nc.gpsimd.tensor_tensor(_nat(o2[:], WW, lo, hi), _nat(to1[:], WW, lo, hi),
                                        _nat(to2[:], WW, lo, hi), mybir.AluOpType.add)
                nc.gpsimd.tensor_tensor(_nat(e2[:], WW, lo, hi), _nat(te1[:], WW, lo, hi),
                                        _nat(te2[:], WW, lo, hi), mybir.AluOpType.add)
                psA = ppool.tile([N, 2 * WW], F32, tag="psA")
                psB = ppool.tile([N, 2 * WW], F32, tag="psB")
                # psB[j, s, w] = wt*(i*sh_dn(o2))[j, s, w], w in [a-2, b+1)
                nc.tensor.matmul(_nat(psB[:], WW, a - 2, b + 1), dn,
                                 _swp(o2[:], WW, a, b + 3), start=True, stop=True)
                # psA[j, s, w] = wt*(i*sh_up(e2))[j, s, w], w in [a, b+3)
                nc.tensor.matmul(_nat(psA[:], WW, a, b + 3), up,
                                 _swp(e2[:], WW, a - 2, b + 1), start=True, stop=True)
                xe_n, xo_n = nxt
                # zero the slivers of the rotated buffers that next stage's
                # wider reads will touch but this stage's writes don't cover
                # (Act engine; no dependency on this stage's compute)
                nc.scalar.memzero(_nat(xe_n[:], WW, a - 4, a - 2))
                nc.scalar.memzero(_nat(xe_n[:], WW, b + 1, b + 5))
                nc.scalar.memzero(_nat(xo_n[:], WW, a - 4, a))
                nc.scalar.memzero(_nat(xo_n[:], WW, b + 3, b + 5))
                nc.vector._custom_dve(
                    CMULA, out=_nat(xe_n[:], WW, a - 2, b + 1), in0=_nat(psB[:], WW, a - 2, b + 1),
                    in1=_nat(e2[:], WW, a - 2, b + 1), s0=1.0, s1=1.0)
                nc.vector._custom_dve(
                    CMULA, out=_nat(xo_n[:], WW, a, b + 3), in0=_nat(psA[:], WW, a, b + 3),
                    in1=_nat(o2[:], WW, a, b + 3), s0=1.0, s1=1.0)
                nxt = [xe, xo]
                xe, xo = xe_n, xo_n
            nc.sync.dma_start(d_oE[:], xe[:])
            nc.sync.dma_start(d_oO[:], xo[:])

    nc.finalize()
    _PROG_A = nc
    return _PROG_A


# ----------------------------------------------------------------------------
# Launch B: chain-apply the 8 group operators to the initial columns
# ----------------------------------------------------------------------------
_PROG_B = None


def _build_program_b():
    global _PROG_B
    if _PROG_B is not None:
        return _PROG_B
    CMULA = _get_cmula()

    import concourse.bacc as bacc
    nc = bacc.Bacc(None, target_bir_lowering=False)
    d_xe = nc.declare_dram_parameter("xe0", [N, 2 * COLS], F16, isOutput=False)
    d_xo = nc.declare_dram_parameter("xo0", [N, 2 * COLS], F16, isOutput=False)
    d_w = [nc.declare_dram_parameter(f"w{g}", [N, 12 * N], F16, isOutput=False)
           for g in range(NCORES)]
    d_pc = nc.declare_dram_parameter("projc", [N, 4], F32, isOutput=False)
    d_out = nc.declare_dram_parameter("out", [N, 2 * COLS], F32, isOutput=True)

    from concourse import tile

    with tile.TileContext(nc) as tc:
        with (tc.tile_pool(name="const", bufs=1) as cpool,
              tc.tile_pool(name="tmp", bufs=2) as tpool,
              tc.tile_pool(name="ps", bufs=2, space="PSUM") as ppool):
            wt = [cpool.tile([N, 12 * N], F16, tag=f"w{g}", name=f"w{g}")
                  for g in range(NCORES)]
            pc = cpool.tile([N, 4], F32, tag="pc")
            outT = cpool.tile([N, 2 * COLS], F32, tag="outT")
            xe = cpool.tile([N, 2 * COLS], F16, tag="xe0")
            xo = cpool.tile([N, 2 * COLS], F16, tag="xo0")
            nc.sync.dma_start(xe[:], d_xe[:])
            nc.sync.dma_start(xo[:], d_xo[:])
            nc.sync.dma_start(pc[:], d_pc[:])
            for g in range(NCORES):
                nc.sync.dma_start(wt[g][:], d_w[g][:])

            for g in range(NCORES):
                # roles: 0 TeeRT 1 TeeIN 2 TeeIP 3 TeoRT 4 TeoIN 5 TeoIP
                #        6 ToeRT 7 ToeIN 8 ToeIP 9 TooRT 10 TooIN 11 TooIP
                def w(r):
                    return wt[g][:, r * N:(r + 1) * N]

                psE = ppool.tile([N, 2 * COLS], F32, tag="psE")
                psO = ppool.tile([N, 2 * COLS], F32, tag="psO")
                xr = (xe[:, 0:COLS], xe[:, COLS:2 * COLS])
                orr = (xo[:, 0:COLS], xo[:, COLS:2 * COLS])
                # XE' = Tee*XE + Teo*XO
                nc.tensor.matmul(psE[:], w(0), xe[:], start=True, stop=False)
                nc.tensor.matmul(psE[:, 0:COLS], w(1), xr[1], start=False, stop=False)
                nc.tensor.matmul(psE[:, COLS:2 * COLS], w(2), xr[0], start=False, stop=False)
                nc.tensor.matmul(psE[:], w(3), xo[:], start=False, stop=False)
                nc.tensor.matmul(psE[:, 0:COLS], w(4), orr[1], start=False, stop=False)
                nc.tensor.matmul(psE[:, COLS:2 * COLS], w(5), orr[0], start=False, stop=True)
                # XO' = Toe*XE + Too*XO
                nc.tensor.matmul(psO[:], w(6), xe[:], start=True, stop=False)
                nc.tensor.matmul(psO[:, 0:COLS], w(7), xr[1], start=False, stop=False)
                nc.tensor.matmul(psO[:, COLS:2 * COLS], w(8), xr[0], start=False, stop=False)
                nc.tensor.matmul(psO[:], w(9), xo[:], start=False, stop=False)
                nc.tensor.matmul(psO[:, 0:COLS], w(10), orr[1], start=False, stop=False)
                nc.tensor.matmul(psO[:, COLS:2 * COLS], w(11), orr[0], start=False, stop=True)
                xe = tpool.tile([N, 2 * COLS], F16, tag="xe")
                xo = tpool.tile([N, 2 * COLS], F16, tag="xo")
                nc.vector.tensor_scalar_mul(xe[:], psE[:], 1.0)
                nc.scalar.copy(xo[:], psO[:])
            # projection: out = f0*XE + f1*XO
            t1 = tpool.tile([N, 2 * COLS], F32, tag="t1")
            t2 = tpool.tile([N, 2 * COLS], F32, tag="t2")
            nc.vector._custom_dve(CMULA, out=_nat(t1[:], COLS, 0, COLS),
                                  in0=_swp(xe[:], COLS, 0, COLS), in1=xe[:],
                                  s0=pc[:, 0:1], s1=pc[:, 1:2])
            nc.vector._custom_dve(CMULA, out=_nat(t2[:], COLS, 0, COLS),
                                  in0=_swp(xo[:], COLS, 0, COLS), in1=xo[:],
                                  s0=pc[:, 2:3], s1=pc[:, 3:4])
            nc.vector.tensor_tensor(outT[:], t1[:], t2[:], mybir.AluOpType.add)
            nc.sync.dma_start(d_out[:], outT[:])

    nc.finalize()
    _PROG_B = nc
    return _PROG_B


# ----------------------------------------------------------------------------
# host-side coefficient construction (same folding as the sequential kernel)
# ----------------------------------------------------------------------------
def _host_inputs(theta_in, theta_even, theta_out):
    theta_in = np.asarray(theta_in, np.float64)
    theta_even = np.asarray(theta_even, np.float64)
    theta_out = np.asarray(theta_out, np.float64)

    aM = math.sqrt(1.0 - IL_MMI)
    bp = aM * math.sqrt(0.5 + IMB)
    bq = aM * math.sqrt(0.5 - IMB)
    B = np.array([[bp, 1j * bq], [1j * bq, bp]], np.complex128)
    aX = math.sqrt(1.0 - IL_X)
    u = aX * math.sqrt(CT)
    vv = aX * math.sqrt(1.0 - CT)

    ph = np.exp(1j * theta_even)  # [255, 128]

    Cs = np.zeros((NSTAGES, N, 2, 2), np.complex128)
    Cs[0, :, :, 0] = B[:, 0][None, :] * ph[0][:, None]
    Cs[0, :, :, 1] = B[:, 1][None, :]
    i = np.arange(1, N - 1)
    a = ph[2 * i - 1]
    b = ph[2 * i]
    T1 = np.zeros((N - 2, N, 2, 2), np.complex128)
    T1[:, :, :, 0] = B[:, 0][None, None, :] * a[:, :, None]
    T1[:, :, :, 1] = B[:, 1][None, None, :]
    T2 = np.zeros_like(T1)
    T2[:, :, :, 0] = B[:, 0][None, None, :] * b[:, :, None]
    T2[:, :, :, 1] = B[:, 1][None, None, :]
    Cs[1:N - 1] = np.einsum("sjab,sjbc->sjac", T2, T1)
    Cs[N - 1, :, :, 0] = B[:, 0][None, :] * ph[2 * N - 3][:, None]
    Cs[N - 1, :, :, 1] = B[:, 1][None, :]
    f0 = np.exp(1j * theta_out) * bp * ph[2 * N - 2]
    f1 = np.exp(1j * theta_out) * (1j * bq)
    Cs[N, :, 0, 0] = f0
    Cs[N, :, 0, 1] = f1

    # fold crossing scalars/corners of K-stage s into stage s+1; the device
    # multiplies the swapped term by WT16 (in the fp16 shift weights), so
    # u_eff * WT16 == vv exactly in f64.
    u_eff = vv / WT16
    dE = np.full(N, u_eff); dE[0] = vv
    dO = np.full(N, u_eff); dO[N - 1] = vv
    Cs[1:N, :, :, 0] *= dE[None, :, None]
    Cs[1:N, :, :, 1] *= dO[None, :, None]

    coefs = [np.ascontiguousarray(x.astype(np.float32)) for x in (
        Cs[:, :, 0, 0].real.T, Cs[:, :, 0, 0].imag.T,
        Cs[:, :, 0, 1].real.T, Cs[:, :, 0, 1].imag.T,
        Cs[:, :, 1, 0].real.T, Cs[:, :, 1, 0].imag.T,
        Cs[:, :, 1, 1].real.T, Cs[:, :, 1, 1].imag.T,
    )]

    din = np.exp(1j * theta_in)
    E0 = np.zeros((N, N), np.complex128)
    O0 = np.zeros((N, N), np.complex128)
    E0[np.arange(N), np.arange(N)] = bp * din
    O0[np.arange(N), np.arange(N)] = 1j * bq * din

    # shift weights (lhsT): psA = S_up @ rhs -> lhsT[j+1, j] = wt (fp16-exact)
    shiftT = np.zeros((N, 2 * N), np.float16)
    shiftT[np.arange(1, N), np.arange(N - 1)] = np.float16(WT16)      # up
    shiftT[np.arange(N - 1), N + np.arange(1, N)] = np.float16(WT16)  # down
    return coefs, E0, O0, shiftT


def _pack(c):  # complex [128, cols] -> f32 [128, 2*cols]
    return np.concatenate([c.real, c.imag], axis=1)


TRACE_DIRS = None
LAST_EXEC_NS = None


def kernel(theta_in, theta_even, theta_out):
    global LAST_EXEC_NS
    from concourse.bass_utils import run_bass_kernel_spmd

    coefs, E0, O0, shiftT = _host_inputs(theta_in, theta_even, theta_out)
    ncA = _build_program_a()
    ncB = _build_program_b()

    # ---- launch A ----
    E0w = np.zeros((N, 2 * WW), np.float32)
    O0w = np.zeros((N, 2 * WW), np.float32)
    E0w[:, B0] = 1.0
    O0w[:, B0 + 1] = 1.0
    sh0 = np.zeros_like(shiftT)
    in_maps_a = []
    for g in range(NCORES):
        m = {"xe0": E0w, "xo0": O0w, "shiftT": shiftT,
             "shiftTl": shiftT if g < NCORES - 1 else sh0}
        for i in range(8):
            m[f"coef{i}"] = np.ascontiguousarray(
                coefs[i][:, g * KSTG:(g + 1) * KSTG])
        in_maps_a.append(m)
    kwA = {}
    if TRACE_DIRS:
        kwA = {"trace": True, "tmpdir": TRACE_DIRS[0]}
    resA = run_bass_kernel_spmd(ncA, in_maps_a, list(range(NCORES)), **kwA)

    # ---- host reshuffle: window -> dense block weights ----
    # final written w-ranges (outside them the rotated buffers hold stale data)
    aF = B0 - 2 * (KSTG - 1)
    bF = B0 + 1 + 2 * (KSTG - 1)
    j = np.arange(N)[:, None]
    w = np.arange(WW)[None, :]
    cidx = 2 * j + w - B0
    valid = (cidx >= 0) & (cidx < 2 * N)
    validE = valid & (w >= aF - 2) & (w < bF + 1)
    validO = valid & (w >= aF) & (w < bF + 3)
    jjE, wwE = np.nonzero(validE)
    ccE = cidx[validE]
    jjO, wwO = np.nonzero(validO)
    ccO = cidx[validO]
    wmaps = []
    for g in range(NCORES):
        oE = resA.results[g]["outE"]
        oO = resA.results[g]["outO"]
        Ew = oE[:, :WW] + 1j * oE[:, WW:]
        Ow = oO[:, :WW] + 1j * oO[:, WW:]
        T = np.zeros((2 * N, 2 * N), np.complex64)
        T[2 * jjE, ccE] = Ew[jjE, wwE]
        T[2 * jjO + 1, ccO] = Ow[jjO, wwO]
        blocks = {"ee": T[0::2, 0::2], "eo": T[0::2, 1::2],
                  "oe": T[1::2, 0::2], "oo": T[1::2, 1::2]}
        roles = []
        for key in ("ee", "eo", "oe", "oo"):
            M = blocks[key]
            roles += [M.real.T, -M.imag.T, M.imag.T]
        wmaps.append(np.ascontiguousarray(
            np.concatenate(roles, axis=1).astype(np.float16)))

    # ---- launch B ----
    pcoef = np.stack([coefs[0][:, NSTAGES - 1], coefs[1][:, NSTAGES - 1],
                      coefs[2][:, NSTAGES - 1], coefs[3][:, NSTAGES - 1]],
                     axis=1).astype(np.float32)
    in_maps_b = []
    for r in range(NCORES):
        cols = slice(r * COLS, (r + 1) * COLS)
        m = {"xe0": _pack(E0[:, cols]).astype(np.float16),
             "xo0": _pack(O0[:, cols]).astype(np.float16),
             "projc": pcoef}
        for g in range(NCORES):
            m[f"w{g}"] = wmaps[g]
        in_maps_b.append(m)
    kwB = {}
    if TRACE_DIRS:
        kwB = {"trace": True, "tmpdir": TRACE_DIRS[1]}
    resB = run_bass_kernel_spmd(ncB, in_maps_b, list(range(NCORES)), **kwB)

    if TRACE_DIRS:
        LAST_EXEC_NS = (resA.exec_time_ns or 0) + (resB.exec_time_ns or 0)

    out = np.zeros((N, N), np.complex64)
    for r in range(NCORES):
        o = resB.results[r]["out"]
        out[:, r * COLS:(r + 1) * COLS] = o[:, :COLS] + 1j * o[:, COLS:]
    return out
